# revision 32
# baseline (speedup 1.0000x reference)
"""Trainium2 Bass kernel for nn_BatchShapingLoss.

Math: loss = sum_{i,j} (pcdf[i,j] - ecdf[i])^2 / n  with pcdf the 1000-point
trapezoid approximation of the Beta(0.6, 0.4) CDF at each sorted value and
ecdf[i] = (i+1)/(n+1).

Restructuring (device-validated 4.2e-4 rel err vs the reference; gate 2e-2):
  * pcdf is an elementwise function F(s); sorting only decides which ecdf row
    a value pairs with, so ranks (count of strictly-smaller keys) replace the
    sort entirely.
  * The 999-term trapezoid sum  F(s) = (s-EPS)^0.6 * sum_k A_k (1-t_k s)^-0.6
    is COMPRESSED to 16 nodes:  F(s) ~= sum_j B_j (1 - tau_j s)^-0.6  with
    8 positive tau (geometric toward 1), 7 negative tau (log-spaced poles that
    absorb the s^0.6 cusp at 0), and the exact k=999 node (tau=1, B=A_999)
    which reproduces the reference's trapezoid blow-up as s->1.  Fitted by
    least squares against the exact trapezoid; max abs error 3.1e-5.
    On device: nodes ride the partition dim (8 per pass, x16 value groups),
    so the whole sum is 2x (Ln+Exp on [128,512]) + two accumulating PE
    matmuls whose +-1 f32r lhsT carries the node signs, one [16,512] PSUM
    bank, an ACT copy to SBUF, one 256B-chunk reshuffle DMA to value layout.
  * Ranks: one compare+accumulate per (column, row-block) unit; 64 units on
    the two engines whose ops are ISA-legal for this (GPSIMD/Pool rejects
    TensorScalarPtr at codegen):
      - DVE (52 units, c3-15): is_lt+accum on uint16 keys (host monotone
        quantization floor(x*65536)); 2-byte packed operands hit the 4x DVE
        perf mode (194ns/unit engine time, ~289ns cadence).
      - ACT (12 units, c0-2): Sign activation with accumulate on a float16
        BITCAST of 14-bit keys (floor(x*16384): bit-pattern order == numeric
        order for positive fp16).  Sign-sum S = #less - #greater counts ties
        as half, EC = (S+513)/1026.
    kt rows c0-2 hold 14-bit keys, the rest 16-bit keys; kp carries f32
    thresholds in both flavors.
  * Epilogue is algebraic: loss*n = sum T^2 - 2 sum T*EC + sum EC^2.
    sum EC^2 is a host-side CONSTANT (ranks are a permutation per column);
    sum T^2 and the two cross dots are single accumulating
    scalar_tensor_tensor ops, so the post-compare tail is ~0.3us + out DMA.
  * Schedule: kb broadcast arrives in column pieces sized/ordered so neither
    compare engine ever starves; dependency pins (add_dep_helper) keep the
    tile scheduler from hoisting the T join or epilogue into stalls.
Host passes kt (column-major keys for the broadcast compare operand), kp
(f32 threshold scalars for both key flavors), and xgp (xg value replicas +
per-partition node constants + sign matrices).
"""

import math

import numpy as np

import concourse.bacc as bacc
import concourse.bass as bass
import concourse.mybir as mybir
import concourse.tile as tile
from concourse.bass_utils import run_bass_kernel_spmd

N = 512  # rows
C_FULL = 128  # total columns
NCORES = 8
CS = C_FULL // NCORES  # 16 columns per core
NB = N // 128  # 4 row blocks
NSIGN = 3   # columns < NSIGN carry 14-bit keys and are ACT sign-compared
F32 = mybir.dt.float32
F32R = mybir.dt.float32r
F16 = mybir.dt.float16
U16 = mybir.dt.uint16

# 16-node compression of the reference trapezoid (see module docstring).
# Node 15 is the exact k=999 trapezoid endpoint term.
TAUS = np.array([
    9.9899899899899902e-01, 9.9699699699699695e-01, 9.9399399399399402e-01,
    9.8298298298298303e-01, 9.5795795795795791e-01, 8.9189189189189189e-01,
    7.2472472472472471e-01, 2.9929929929929933e-01, -2.9999999999999999e-01,
    -2.4980495329668129e+00, -2.0800838230519037e+01, -1.7320508075688775e+02,
    -1.4422495703074076e+03, -1.2009369551760035e+04, -1.0000000000000000e+05,
    1.0000000000000000e+00])
WTS = np.array([
    3.3413894642732328e-04, 9.1904509968264035e-04, 1.3621900590513583e-03,
    5.3975794544086388e-03, 1.2320754778031927e-02, 3.4949489757262572e-02,
    1.1587961165884496e-01, 8.4242771913389780e-01, -6.6557499152776400e-01,
    -2.4558635524108757e-01, -7.3587817100481373e-02, -2.0703846243494806e-02,
    -6.1519292764726629e-03, -1.3229548982430946e-03, -7.6353083657170156e-04,
    1.5151686259072211e-04])

# xgp packed layout (f32 columns)
XG0 = 0
TNA, TNB, LBA, LBB = 512, 513, 514, 515
G0 = 516
XGP_F = 548

# kb DMA piece order: c2 first (DVE/GPSIMD start there), xgp is interleaved
# by the emitter, ACT's c0-1 arrive well before its pcdf chain finishes
KB_PIECES = ((3, 4), (4, 6), (6, 9), (0, 3), (9, 12), (12, 16))
# xgp rides after this many kb pieces in the transfer queue
XGP_SLOT = 2

# sum_i (i/513)^2 for i=1..512, per column (the ecdf^2 term is rank-free)
ECC_PER_COL = float(np.sum((np.arange(1, N + 1, dtype=np.float64) / (N + 1)) ** 2))


def _host_constants():
    """Per-partition node constants for the two ACT passes.

    Partition k = g*8 + j serves node nid = r*8 + j in pass r (g = value
    group).  Returns tneg[r][128], lnb[r][128], G[r][128, 16]."""
    j = np.arange(128) % 8
    g = np.arange(128) // 8
    tneg, lnb, G = [], [], []
    for r in range(2):
        nid = r * 8 + j
        tneg.append((-TAUS[nid]).astype(np.float32))
        lnb.append(np.log(np.abs(WTS[nid])).astype(np.float32))
        Gm = np.zeros((128, 16), np.float32)
        Gm[np.arange(128), g] = np.sign(WTS[nid]).astype(np.float32)
        G.append(Gm)
    return tneg, lnb, G


def _build_body(ctx, tc, kt_d, kp_d, xgp_d, out_d):
    from concourse.tile_rust import add_dep_helper

    nc = tc.nc
    AF = mybir.ActivationFunctionType
    OP = mybir.AluOpType

    singles = ctx.enter_context(tc.tile_pool(name="singles", bufs=1))
    l_pool = ctx.enter_context(tc.tile_pool(name="lt", bufs=2))
    e_pool = ctx.enter_context(tc.tile_pool(name="et", bufs=2))
    ps_pool = ctx.enter_context(tc.tile_pool(name="ps", bufs=1, space="PSUM"))

    kb = singles.tile([128, CS, N], U16)
    kp_s = singles.tile([128, 2 * CS * NB], F32)
    xgp_s = singles.tile([128, XGP_F], F32)
    gm_s = singles.tile([128, 32], F32R)

    # All input DMAs issue from SP in transfer-priority order: thresholds,
    # first key columns, the ACT operand tensor, remaining key columns.
    def kb_piece(c0, c1):
        nc.sync.dma_start(
            out=kb[:, c0:c1, :],
            in_=bass.AP(tensor=kt_d.tensor, offset=c0 * N,
                        ap=[[0, 128], [1, (c1 - c0) * N]]),
        )
    kb_piece(*KB_PIECES[0])
    nc.sync.dma_start(out=kp_s, in_=kp_d)
    for c0, c1 in KB_PIECES[1:XGP_SLOT]:
        kb_piece(c0, c1)
    nc.sync.dma_start(out=xgp_s, in_=xgp_d)
    for c0, c1 in KB_PIECES[XGP_SLOT:]:
        kb_piece(c0, c1)

    # Tiny warm-up activation with no DMA dependency: pulls the one
    # ACT_TABLE_LOAD (natural_log_exp_and_others) to t~0.3us.
    warm_s = singles.tile([1, 1], F32)
    nc.vector.memset(warm_s, 0.5)
    nc.scalar.activation(out=warm_s, in_=warm_s, func=AF.Exp, bias=0.0, scale=1.0)

    xg = xgp_s[:, XG0:XG0 + 512]
    tneg = (xgp_s[:, TNA:TNA + 1], xgp_s[:, TNB:TNB + 1])
    lnb = (xgp_s[:, LBA:LBA + 1], xgp_s[:, LBB:LBB + 1])
    G = (gm_s[:, 0:16], gm_s[:, 16:32])
    kp = kp_s[:, :CS * NB].rearrange("p (c b) -> p c b", b=NB)
    kpa = kp_s[:, CS * NB:].rearrange("p (c b) -> p c b", b=NB)

    # ---- pcdf: 16-node sum = 2x (Ln + Exp) + 2 accumulating matmuls ----
    # fp32r matmul operands must be PRODUCED as f32r: round the +-1 sign
    # matrices through a tiny ACT copy (the DMA delivers them as f32).
    # First so the matmuls fire as soon as each Exp lands.
    nc.scalar.activation(out=gm_s, in_=xgp_s[:, G0:G0 + 32], func=AF.Copy)
    ps = ps_pool.tile([16, 512], F32)
    for r in range(2):
        L = l_pool.tile([128, 512], F32)
        nc.scalar.activation(out=L, in_=xg, func=AF.Ln, bias=1.0, scale=tneg[r])
        E = e_pool.tile([128, 512], F32R)
        nc.scalar.activation(out=E, in_=L, func=AF.Exp, bias=lnb[r], scale=-0.6)
        nc.tensor.matmul(ps[:, :], G[r], E, start=(r == 0), stop=(r == 1))

    # ---- ranks + join, on the two engines that can legally run them ----
    # (the GPSIMD/Pool engine rejects TensorScalarPtr at ISA level, so it
    # contributes nothing here)
    # Sign units (c0-2, ACT): sign-sum #less - #greater via Sign with
    #   accumulate on the fp16 bitcast of 14-bit keys; ties count half.
    # Count units (c3-15, DVE): #{q : key[q,c] < key[b*128+p, c]} via
    #   is_lt+accum in uint16 (4x DVE perf mode).
    # Epilogue dot slices: [:, :3, :] (sign) and [:, 3:, :] (count).
    junk_dve = singles.tile([128, N], U16)
    junk_act = singles.tile([128, N], F32)
    R = singles.tile([128, CS, NB], F32)
    SQ = singles.tile([128, CS, NB], F32)
    T2 = singles.tile([128, CS, NB], F32)
    acc = singles.tile([128, 3], F32)
    T = singles.tile([128, CS, NB], F32)

    def unit(c, b):
        if c < NSIGN:
            return nc.scalar.activation(
                out=junk_act,
                in_=kb[:, c, :].bitcast(F16),
                func=AF.Sign,
                bias=kpa[:, c, b:b + 1],
                scale=-1.0,
                accum_out=R[:, c, b:b + 1],
            )
        return nc.vector.tensor_scalar(
            out=junk_dve,
            in0=kb[:, c, :],
            scalar1=kp[:, c, b:b + 1],
            scalar2=None,
            op0=OP.is_lt,
            op1=OP.add,  # reduce op for accum_out
            accum_out=R[:, c, b:b + 1],
        )

    psb = singles.tile([16, 512], F32)
    T_flat = T.rearrange("p c b -> p (c b)")

    # ACT: one sign unit (hides the matmul latency), then the PSUM -> SBUF
    # copy for the T join, then the remaining sign compares.
    psb_inst = None
    for c in range(NSIGN):
        for b in range(NB):
            s = unit(c, b)
            if psb_inst is None:
                psb_inst = nc.scalar.activation(out=psb, in_=ps, func=AF.Copy)
                add_dep_helper(psb_inst.ins, s.ins, sync=False,
                               reason="psb after the first sign unit")
                nc.sync.dma_start(
                    out=T_flat,
                    in_=psb.rearrange("g (h u) -> g h u", u=64),
                )
            elif s is not None and psb_inst is not None and (c, b) == (0, 1):
                add_dep_helper(s.ins, psb_inst.ins, sync=False,
                               reason="sign stream resumes after psb")

    # DVE: the count compares, with the T^2 dot slotted late enough that the
    # joined T has landed (emitted after the join: deps follow emission order)
    last_dve = None
    for c in range(3, 13):
        for b in range(NB):
            last_dve = unit(c, b)
    t2_inst = nc.vector.scalar_tensor_tensor(
        out=T2.rearrange("p c b -> p (c b)"), in0=T_flat, scalar=1.0,
        in1=T_flat, op0=OP.mult, op1=OP.mult,
        accum_out=acc[:, 2:3],
    )
    add_dep_helper(t2_inst.ins, last_dve.ins, sync=False,
                   reason="T^2 late in the DVE stream")
    for c in range(13, CS):
        for b in range(NB):
            last_dve = unit(c, b)

    # ---- epilogue: two cross dots; EC^2 is a host constant ----
    # loss*n = sum T^2 - (2/513) sum T*(R+1)  [count units]
    #                  - (2/1026) sum T*(S+513) [sign units]  + ECC
    dot1 = nc.vector.scalar_tensor_tensor(
        out=SQ[:, NSIGN:, :], in0=R[:, NSIGN:, :], scalar=1.0,
        in1=T[:, NSIGN:, :], op0=OP.add, op1=OP.mult,
        accum_out=acc[:, 0:1],
    )
    add_dep_helper(dot1.ins, last_dve.ins, sync=False,
                   reason="epilogue after the DVE compare stream")
    dot2 = nc.vector.scalar_tensor_tensor(
        out=SQ[:, :NSIGN, :], in0=R[:, :NSIGN, :], scalar=513.0,
        in1=T[:, :NSIGN, :], op0=OP.add, op1=OP.mult,
        accum_out=acc[:, 1:2],
    )
    add_dep_helper(dot2.ins, dot1.ins, sync=False,
                   reason="epilogue order")
    nc.sync.dma_start(out=out_d, in_=acc)


import contextlib


@contextlib.contextmanager
def _patched_act_tables():
    """Scoped patch: force the act-table pass to use
    natural_log_exp_and_others (which has Ln, Exp, Sign, Copy, Square) so the
    kernel pays exactly one table load.  Only the eligibility sets are
    filtered, and only while compiling this module's kernel."""
    import concourse.bacc as _bacc
    import concourse.hw_specs as _hw

    orig_hw = _hw.get_activation_tables
    orig_bacc = _bacc.get_activation_tables

    def patched(arch):
        tabs = orig_hw(arch)
        return {
            name: (funcs if name == "natural_log_exp_and_others" else set())
            for name, funcs in tabs.items()
        }

    _bacc.get_activation_tables = patched
    try:
        yield
    finally:
        _bacc.get_activation_tables = orig_bacc


def build_nc():
    nc = bacc.Bacc(
        "TRN2",
        target_bir_lowering=False,
        debug=False,
        enable_asserts=False,
        num_devices=NCORES,
    )
    kt_d = nc.dram_tensor("kt", [CS, N], U16, kind="ExternalInput").ap()
    kp_d = nc.dram_tensor("kp", [128, 2 * CS * NB], F32, kind="ExternalInput").ap()
    xgp_d = nc.dram_tensor("xgp", [128, XGP_F], F32, kind="ExternalInput").ap()
    out_d = nc.dram_tensor("out", [128, 3], F32, kind="ExternalOutput").ap()

    from contextlib import ExitStack

    with _patched_act_tables():
        with ExitStack() as ctx:
            tc = ctx.enter_context(tile.TileContext(nc))
            _build_body(ctx, tc, kt_d, kp_d, xgp_d, out_d)
        nc.compile()
    return nc


_NC_CACHE = None


def _get_nc():
    global _NC_CACHE
    if _NC_CACHE is None:
        _NC_CACHE = build_nc()
    return _NC_CACHE


def _pcb_layout(a):
    """[512, 16] -> [128, 64] in (p, c, b) order: slot w = c*4+b holds row
    b*128+p."""
    return np.ascontiguousarray(
        a.reshape(NB, 128, CS).transpose(1, 2, 0)
    ).reshape(128, CS * NB)


def _make_in_maps(x):
    tneg, lnb, G = _host_constants()
    x64 = x.astype(np.float64)
    keys16 = np.minimum(np.floor(x64 * 65536.0), 65535.0).astype(np.uint16)
    keys14 = np.minimum(np.floor(x64 * 16384.0), 16383.0).astype(np.uint16)
    in_maps = []
    for m in range(NCORES):
        sl = slice(m * CS, (m + 1) * CS)
        xs = x[:, sl].astype(np.float32)
        k16 = keys16[:, sl]
        k14 = keys14[:, sl]
        # kt rows: 14-bit keys for the sign columns (c<3), 16-bit elsewhere
        kt = np.ascontiguousarray(k16.T)
        kt[:NSIGN, :] = k14.T[:NSIGN, :]
        # threshold scalars for count units: f32 of the SAME integer key that
        # kt carries for that column (16-bit below ACT_C0, 14-bit above);
        # for sign units: f32 value of the fp16 whose bits are the 14-bit key
        kmix = k16.copy()
        kmix[:, :NSIGN] = k14[:, :NSIGN]
        kpc = _pcb_layout(kmix).astype(np.float32)
        kpa = np.ascontiguousarray(_pcb_layout(k14)).view(np.float16).astype(np.float32)
        kpf = np.concatenate([kpc, kpa], axis=1)
        # value enumeration v = p*64 + w; groups of 512, replicated x8 nodes
        xp = _pcb_layout(xs)
        xg = np.broadcast_to(xp.reshape(16, 1, 512), (16, 8, 512)).reshape(128, 512)
        xgp = np.empty((128, XGP_F), np.float32)
        xgp[:, XG0:XG0 + 512] = xg
        xgp[:, TNA] = tneg[0]
        xgp[:, TNB] = tneg[1]
        xgp[:, LBA] = lnb[0]
        xgp[:, LBB] = lnb[1]
        xgp[:, G0:G0 + 16] = G[0]
        xgp[:, G0 + 16:G0 + 32] = G[1]
        in_maps.append({
            "kt": kt,
            "kp": np.ascontiguousarray(kpf),
            "xgp": np.ascontiguousarray(xgp),
        })
    return in_maps


def kernel(x: np.ndarray) -> np.ndarray:
    x = np.ascontiguousarray(np.asarray(x, dtype=np.float32))
    assert x.shape == (N, C_FULL)
    nc = _get_nc()
    in_maps = _make_in_maps(x)
    res = run_bass_kernel_spmd(nc, in_maps, core_ids=list(range(NCORES)))
    total = 0.0
    for r in res.results:
        a = r["out"].astype(np.float64)
        dot_u16, dot_act, sum_t2 = a[:, 0].sum(), a[:, 1].sum(), a[:, 2].sum()
        total += sum_t2 - 2.0 * (dot_u16 / (N + 1) + dot_act / (2 * (N + 1)))
    total += C_FULL * ECC_PER_COL
    return np.array(total / N, dtype=np.float32)


# revision 40
# speedup vs baseline: 1.0023x; 1.0023x over previous
"""Trainium2 Bass kernel for nn_BatchShapingLoss.

Math: loss = sum_{i,j} (pcdf[i,j] - ecdf[i])^2 / n  with pcdf the 1000-point
trapezoid approximation of the Beta(0.6, 0.4) CDF at each sorted value and
ecdf[i] = (i+1)/(n+1).

Restructuring (device-validated 4.2e-4 rel err vs the reference; gate 2e-2):
  * pcdf is an elementwise function F(s); sorting only decides which ecdf row
    a value pairs with, so ranks (count of strictly-smaller keys) replace the
    sort entirely.
  * The 999-term trapezoid sum  F(s) = (s-EPS)^0.6 * sum_k A_k (1-t_k s)^-0.6
    is COMPRESSED to 16 nodes:  F(s) ~= sum_j B_j (1 - tau_j s)^-0.6  with
    8 positive tau (geometric toward 1), 7 negative tau (log-spaced poles that
    absorb the s^0.6 cusp at 0), and the exact k=999 node (tau=1, B=A_999)
    which reproduces the reference's trapezoid blow-up as s->1.  Fitted by
    least squares against the exact trapezoid; max abs error 3.1e-5.
    On device: nodes ride the partition dim (8 per pass, x16 value groups),
    so the whole sum is 2x (Ln+Exp on [128,512]) + two accumulating PE
    matmuls whose +-1 f32r lhsT carries the node signs, one [16,512] PSUM
    bank, an ACT copy to SBUF, one 256B-chunk reshuffle DMA to value layout.
  * Ranks: one compare+accumulate per (column, row-block) unit; 64 units on
    the two engines whose ops are ISA-legal for this (GPSIMD/Pool rejects
    TensorScalarPtr at codegen):
      - DVE (52 units, c3-15): is_lt+accum on uint16 keys (host monotone
        quantization floor(x*65536)); 2-byte packed operands hit the 4x DVE
        perf mode (194ns/unit engine time, ~289ns cadence).
      - ACT (12 units, c0-2): Sign activation with accumulate on a float16
        BITCAST of 14-bit keys (floor(x*16384): bit-pattern order == numeric
        order for positive fp16).  Sign-sum S = #less - #greater counts ties
        as half, EC = (S+513)/1026.
    kt rows c0-2 hold 14-bit keys, the rest 16-bit keys; kp carries f32
    thresholds in both flavors.
  * Epilogue is algebraic: loss*n = sum T^2 - 2 sum T*EC + sum EC^2.
    sum EC^2 is a host-side CONSTANT (ranks are a permutation per column);
    sum T^2 and the two cross dots are single accumulating
    scalar_tensor_tensor ops, so the post-compare tail is ~0.3us + out DMA.
  * Schedule: kb broadcast arrives in column pieces sized/ordered so neither
    compare engine ever starves; dependency pins (add_dep_helper) keep the
    tile scheduler from hoisting the T join or epilogue into stalls.
Host passes kt (column-major keys for the broadcast compare operand), kp
(f32 threshold scalars for both key flavors), and xgp (xg value replicas +
per-partition node constants + sign matrices).
"""

import math

import numpy as np

import concourse.bacc as bacc
import concourse.bass as bass
import concourse.mybir as mybir
import concourse.tile as tile
from concourse.bass_utils import run_bass_kernel_spmd

N = 512  # rows
C_FULL = 128  # total columns
NCORES = 8
CS = C_FULL // NCORES  # 16 columns per core
NB = N // 128  # 4 row blocks
NSIGN = 3   # columns < NSIGN carry 14-bit keys and are ACT sign-compared
F32 = mybir.dt.float32
F32R = mybir.dt.float32r
F16 = mybir.dt.float16
U16 = mybir.dt.uint16

# 16-node compression of the reference trapezoid (see module docstring).
# Node 15 is the exact k=999 trapezoid endpoint term.
TAUS = np.array([
    9.9899899899899902e-01, 9.9699699699699695e-01, 9.9399399399399402e-01,
    9.8298298298298303e-01, 9.5795795795795791e-01, 8.9189189189189189e-01,
    7.2472472472472471e-01, 2.9929929929929933e-01, -2.9999999999999999e-01,
    -2.4980495329668129e+00, -2.0800838230519037e+01, -1.7320508075688775e+02,
    -1.4422495703074076e+03, -1.2009369551760035e+04, -1.0000000000000000e+05,
    1.0000000000000000e+00])
WTS = np.array([
    3.3413894642732328e-04, 9.1904509968264035e-04, 1.3621900590513583e-03,
    5.3975794544086388e-03, 1.2320754778031927e-02, 3.4949489757262572e-02,
    1.1587961165884496e-01, 8.4242771913389780e-01, -6.6557499152776400e-01,
    -2.4558635524108757e-01, -7.3587817100481373e-02, -2.0703846243494806e-02,
    -6.1519292764726629e-03, -1.3229548982430946e-03, -7.6353083657170156e-04,
    1.5151686259072211e-04])

# xgp packed layout (f32 columns)
XG0 = 0
TNA, TNB, LBA, LBB = 512, 513, 514, 515
G0 = 516
XGP_F = 548

# kb DMA piece order: c2 first (DVE/GPSIMD start there), xgp is interleaved
# by the emitter, ACT's c0-1 arrive well before its pcdf chain finishes
KB_PIECES = ((3, 4), (4, 6), (6, 9), (0, 3), (9, 16))
# xgp rides after this many kb pieces in the transfer queue
XGP_SLOT = 2

# sum_i (i/513)^2 for i=1..512, per column (the ecdf^2 term is rank-free)
ECC_PER_COL = float(np.sum((np.arange(1, N + 1, dtype=np.float64) / (N + 1)) ** 2))


def _host_constants():
    """Per-partition node constants for the two ACT passes.

    Partition k = g*8 + j serves node nid = r*8 + j in pass r (g = value
    group).  Returns tneg[r][128], lnb[r][128], G[r][128, 16]."""
    j = np.arange(128) % 8
    g = np.arange(128) // 8
    tneg, lnb, G = [], [], []
    for r in range(2):
        nid = r * 8 + j
        tneg.append((-TAUS[nid]).astype(np.float32))
        lnb.append(np.log(np.abs(WTS[nid])).astype(np.float32))
        Gm = np.zeros((128, 16), np.float32)
        Gm[np.arange(128), g] = np.sign(WTS[nid]).astype(np.float32)
        G.append(Gm)
    return tneg, lnb, G


def _build_body(ctx, tc, kt_d, kp_d, xgp_d, out_d):
    from concourse.tile_rust import add_dep_helper

    nc = tc.nc
    AF = mybir.ActivationFunctionType
    OP = mybir.AluOpType

    singles = ctx.enter_context(tc.tile_pool(name="singles", bufs=1))
    l_pool = ctx.enter_context(tc.tile_pool(name="lt", bufs=2))
    e_pool = ctx.enter_context(tc.tile_pool(name="et", bufs=2))
    ps_pool = ctx.enter_context(tc.tile_pool(name="ps", bufs=1, space="PSUM"))

    kb = singles.tile([128, CS, N], U16)
    kp_s = singles.tile([128, 2 * CS * NB], F32)
    xgp_s = singles.tile([128, XGP_F], F32)
    gm_s = singles.tile([128, 32], F32R)

    # All input DMAs issue from SP in transfer-priority order: thresholds,
    # first key columns, the ACT operand tensor, remaining key columns.
    def kb_piece(c0, c1):
        nc.sync.dma_start(
            out=kb[:, c0:c1, :],
            in_=bass.AP(tensor=kt_d.tensor, offset=c0 * N,
                        ap=[[0, 128], [1, (c1 - c0) * N]]),
        )
    kb_piece(*KB_PIECES[0])
    nc.sync.dma_start(out=kp_s, in_=kp_d)
    for c0, c1 in KB_PIECES[1:XGP_SLOT]:
        kb_piece(c0, c1)
    nc.sync.dma_start(out=xgp_s, in_=xgp_d)
    for c0, c1 in KB_PIECES[XGP_SLOT:]:
        kb_piece(c0, c1)

    # Tiny warm-up activation with no DMA dependency: pulls the one
    # ACT_TABLE_LOAD (natural_log_exp_and_others) to t~0.3us.
    warm_s = singles.tile([1, 1], F32)
    nc.vector.memset(warm_s, 0.5)
    nc.scalar.activation(out=warm_s, in_=warm_s, func=AF.Exp, bias=0.0, scale=1.0)

    xg = xgp_s[:, XG0:XG0 + 512]
    tneg = (xgp_s[:, TNA:TNA + 1], xgp_s[:, TNB:TNB + 1])
    lnb = (xgp_s[:, LBA:LBA + 1], xgp_s[:, LBB:LBB + 1])
    G = (gm_s[:, 0:16], gm_s[:, 16:32])
    kp = kp_s[:, :CS * NB].rearrange("p (c b) -> p c b", b=NB)
    kpa = kp_s[:, CS * NB:].rearrange("p (c b) -> p c b", b=NB)

    # ---- pcdf: 16-node sum = 2x (Ln + Exp) + 2 accumulating matmuls ----
    # fp32r matmul operands must be PRODUCED as f32r: round the +-1 sign
    # matrices through a tiny ACT copy (the DMA delivers them as f32).
    # First so the matmuls fire as soon as each Exp lands.
    nc.scalar.activation(out=gm_s, in_=xgp_s[:, G0:G0 + 32], func=AF.Copy)
    ps = ps_pool.tile([16, 512], F32)
    for r in range(2):
        L = l_pool.tile([128, 512], F32)
        nc.scalar.activation(out=L, in_=xg, func=AF.Ln, bias=1.0, scale=tneg[r])
        E = e_pool.tile([128, 512], F32R)
        nc.scalar.activation(out=E, in_=L, func=AF.Exp, bias=lnb[r], scale=-0.6)
        nc.tensor.matmul(ps[:, :], G[r], E, start=(r == 0), stop=(r == 1))

    # ---- ranks + join, on the two engines that can legally run them ----
    # (the GPSIMD/Pool engine rejects TensorScalarPtr at ISA level, so it
    # contributes nothing here)
    # Sign units (c0-2, ACT): sign-sum #less - #greater via Sign with
    #   accumulate on the fp16 bitcast of 14-bit keys; ties count half.
    # Count units (c3-15, DVE): #{q : key[q,c] < key[b*128+p, c]} via
    #   is_lt+accum in uint16 (4x DVE perf mode).
    # Epilogue dot slices: [:, :3, :] (sign) and [:, 3:, :] (count).
    junk_dve = singles.tile([128, N], U16)
    junk_act = singles.tile([128, N], F32)
    R = singles.tile([128, CS, NB], F32)
    SQ = singles.tile([128, CS, NB], F32)
    T2 = singles.tile([128, CS, NB], F32)
    acc = singles.tile([128, 3], F32)
    T = singles.tile([128, CS, NB], F32)

    def unit(c, b):
        if c < NSIGN:
            return nc.scalar.activation(
                out=junk_act,
                in_=kb[:, c, :].bitcast(F16),
                func=AF.Sign,
                bias=kpa[:, c, b:b + 1],
                scale=-1.0,
                accum_out=R[:, c, b:b + 1],
            )
        return nc.vector.tensor_scalar(
            out=junk_dve,
            in0=kb[:, c, :],
            scalar1=kp[:, c, b:b + 1],
            scalar2=None,
            op0=OP.is_lt,
            op1=OP.add,  # reduce op for accum_out
            accum_out=R[:, c, b:b + 1],
        )

    psb = singles.tile([16, 512], F32)
    T_flat = T.rearrange("p c b -> p (c b)")

    # ACT: one sign unit (hides the matmul latency), then the PSUM -> SBUF
    # copy for the T join, then the remaining sign compares.
    psb_inst = None
    for c in range(NSIGN):
        for b in range(NB):
            s = unit(c, b)
            if psb_inst is None:
                psb_inst = nc.scalar.activation(out=psb, in_=ps, func=AF.Copy)
                add_dep_helper(psb_inst.ins, s.ins, sync=False,
                               reason="psb after the first sign unit")
                nc.sync.dma_start(
                    out=T_flat,
                    in_=psb.rearrange("g (h u) -> g h u", u=64),
                )
            elif s is not None and psb_inst is not None and (c, b) == (0, 1):
                add_dep_helper(s.ins, psb_inst.ins, sync=False,
                               reason="sign stream resumes after psb")

    # DVE: the count compares, with the T^2 dot slotted late enough that the
    # joined T has landed (emitted after the join: deps follow emission order)
    last_dve = None
    for c in range(3, 13):
        for b in range(NB):
            last_dve = unit(c, b)
    t2_inst = nc.vector.scalar_tensor_tensor(
        out=T2.rearrange("p c b -> p (c b)"), in0=T_flat, scalar=1.0,
        in1=T_flat, op0=OP.mult, op1=OP.mult,
        accum_out=acc[:, 2:3],
    )
    add_dep_helper(t2_inst.ins, last_dve.ins, sync=False,
                   reason="T^2 late in the DVE stream")
    for c in range(13, CS):
        for b in range(NB):
            last_dve = unit(c, b)

    # ---- epilogue: two cross dots; EC^2 is a host constant ----
    # loss*n = sum T^2 - (2/513) sum T*(R+1)  [count units]
    #                  - (2/1026) sum T*(S+513) [sign units]  + ECC
    dot1 = nc.vector.scalar_tensor_tensor(
        out=SQ[:, NSIGN:, :], in0=R[:, NSIGN:, :], scalar=1.0,
        in1=T[:, NSIGN:, :], op0=OP.add, op1=OP.mult,
        accum_out=acc[:, 0:1],
    )
    add_dep_helper(dot1.ins, last_dve.ins, sync=False,
                   reason="epilogue after the DVE compare stream")
    dot2 = nc.vector.scalar_tensor_tensor(
        out=SQ[:, :NSIGN, :], in0=R[:, :NSIGN, :], scalar=513.0,
        in1=T[:, :NSIGN, :], op0=OP.add, op1=OP.mult,
        accum_out=acc[:, 1:2],
    )
    add_dep_helper(dot2.ins, dot1.ins, sync=False,
                   reason="epilogue order")
    nc.sync.dma_start(out=out_d, in_=acc)


import contextlib


@contextlib.contextmanager
def _patched_act_tables():
    """Scoped patch: force the act-table pass to use
    natural_log_exp_and_others (which has Ln, Exp, Sign, Copy, Square) so the
    kernel pays exactly one table load.  Only the eligibility sets are
    filtered, and only while compiling this module's kernel."""
    import concourse.bacc as _bacc
    import concourse.hw_specs as _hw

    orig_hw = _hw.get_activation_tables
    orig_bacc = _bacc.get_activation_tables

    def patched(arch):
        tabs = orig_hw(arch)
        return {
            name: (funcs if name == "natural_log_exp_and_others" else set())
            for name, funcs in tabs.items()
        }

    _bacc.get_activation_tables = patched
    try:
        yield
    finally:
        _bacc.get_activation_tables = orig_bacc


def build_nc():
    nc = bacc.Bacc(
        "TRN2",
        target_bir_lowering=False,
        debug=False,
        enable_asserts=False,
        num_devices=NCORES,
    )
    kt_d = nc.dram_tensor("kt", [CS, N], U16, kind="ExternalInput").ap()
    kp_d = nc.dram_tensor("kp", [128, 2 * CS * NB], F32, kind="ExternalInput").ap()
    xgp_d = nc.dram_tensor("xgp", [128, XGP_F], F32, kind="ExternalInput").ap()
    out_d = nc.dram_tensor("out", [128, 3], F32, kind="ExternalOutput").ap()

    from contextlib import ExitStack

    with _patched_act_tables():
        with ExitStack() as ctx:
            tc = ctx.enter_context(tile.TileContext(nc))
            _build_body(ctx, tc, kt_d, kp_d, xgp_d, out_d)
        nc.compile()
    return nc


_NC_CACHE = None


def _get_nc():
    global _NC_CACHE
    if _NC_CACHE is None:
        _NC_CACHE = build_nc()
    return _NC_CACHE


def _pcb_layout(a):
    """[512, 16] -> [128, 64] in (p, c, b) order: slot w = c*4+b holds row
    b*128+p."""
    return np.ascontiguousarray(
        a.reshape(NB, 128, CS).transpose(1, 2, 0)
    ).reshape(128, CS * NB)


def _make_in_maps(x):
    tneg, lnb, G = _host_constants()
    x64 = x.astype(np.float64)
    keys16 = np.minimum(np.floor(x64 * 65536.0), 65535.0).astype(np.uint16)
    keys14 = np.minimum(np.floor(x64 * 16384.0), 16383.0).astype(np.uint16)
    in_maps = []
    for m in range(NCORES):
        sl = slice(m * CS, (m + 1) * CS)
        xs = x[:, sl].astype(np.float32)
        k16 = keys16[:, sl]
        k14 = keys14[:, sl]
        # kt rows: 14-bit keys for the sign columns (c<3), 16-bit elsewhere
        kt = np.ascontiguousarray(k16.T)
        kt[:NSIGN, :] = k14.T[:NSIGN, :]
        # threshold scalars for count units: f32 of the SAME integer key that
        # kt carries for that column (16-bit below ACT_C0, 14-bit above);
        # for sign units: f32 value of the fp16 whose bits are the 14-bit key
        kmix = k16.copy()
        kmix[:, :NSIGN] = k14[:, :NSIGN]
        kpc = _pcb_layout(kmix).astype(np.float32)
        kpa = np.ascontiguousarray(_pcb_layout(k14)).view(np.float16).astype(np.float32)
        kpf = np.concatenate([kpc, kpa], axis=1)
        # value enumeration v = p*64 + w; groups of 512, replicated x8 nodes
        xp = _pcb_layout(xs)
        xg = np.broadcast_to(xp.reshape(16, 1, 512), (16, 8, 512)).reshape(128, 512)
        xgp = np.empty((128, XGP_F), np.float32)
        xgp[:, XG0:XG0 + 512] = xg
        xgp[:, TNA] = tneg[0]
        xgp[:, TNB] = tneg[1]
        xgp[:, LBA] = lnb[0]
        xgp[:, LBB] = lnb[1]
        xgp[:, G0:G0 + 16] = G[0]
        xgp[:, G0 + 16:G0 + 32] = G[1]
        in_maps.append({
            "kt": kt,
            "kp": np.ascontiguousarray(kpf),
            "xgp": np.ascontiguousarray(xgp),
        })
    return in_maps


def kernel(x: np.ndarray) -> np.ndarray:
    x = np.ascontiguousarray(np.asarray(x, dtype=np.float32))
    assert x.shape == (N, C_FULL)
    nc = _get_nc()
    in_maps = _make_in_maps(x)
    res = run_bass_kernel_spmd(nc, in_maps, core_ids=list(range(NCORES)))
    total = 0.0
    for r in res.results:
        a = r["out"].astype(np.float64)
        dot_u16, dot_act, sum_t2 = a[:, 0].sum(), a[:, 1].sum(), a[:, 2].sum()
        total += sum_t2 - 2.0 * (dot_u16 / (N + 1) + dot_act / (2 * (N + 1)))
    total += C_FULL * ECC_PER_COL
    return np.array(total / N, dtype=np.float32)
